# revision 13
# baseline (speedup 1.0000x reference)
"""Cross-attention kernel for Trainium2, SPMD over 8 NeuronCores.

Problem: B=4, N=2048, C=1024 fp32.
  q = event_f @ Wq + bq ; k = img_f @ Wk + bk ; v = img_f @ Wv + bv
  out = softmax(q k^T / sqrt(C)) v

Sharding: core i = (batch b = i//2, half h = i%2). Each core projects q^T for
its 1024 query rows and k^T/v for its 1024 KEY rows, exchanges the k/v halves
with its pair partner (two early pairwise AllGathers, k first), and overlaps
the exchange with attention against its LOCAL keys.  The partner half is then
read back (tc.If on partition_id&1 picks the partner slot) and attention over
the REMOTE keys finishes the job:

  out[q,:] = (sum_loc p_l v_l + sum_rem p_r v_r) / (S_l + S_r)

Softmax is key-order invariant, so local/remote key splitting is safe.  No
max-subtraction (logits are N(0,1)); exp in fp16.

Layouts (all fat-row, 16KB contiguous per partition, for large DMA packets):
  host ships x^T / W tiled as [128p, 8t, 1024] fp16 with c_in = t*128+p.
  - q^T/k^T[c,n]: stationary = W[:, t, co*128:+128], moving = x^T tiles
  - v[n,c]:       stationary = img^T[:, t, nr*128:+128], moving = Wv
  - s^T[k,q]:     stationary = k^T[:, t, kt*128:+128], moving = q^T
  - out[q,c]:     stationary = exp[:, kt, q4*128:+128], moving = v
  - sums: DVE reduces exp over kt -> red[128,1024]; one matmul per q4 block
    (red_blk^T @ ones) gives per-query partial sums.
All matmul operands fp16, PSUM fp32, output fp32.
"""

import json

import numpy as np

B, N, C = 4, 2048, 1024
NQ = N // 2          # query rows per core
CT = C // 128        # contraction tiles
KT2 = NQ // 128      # key tiles per half
Q4 = NQ // 128       # query tiles
SCALE = 1.0 / np.sqrt(C)

_CACHE = {}


# ---------------------------------------------------------------------------
# Walrus in this container rejects >1 embedded sem-wait per instruction
# ("Too many sync wait commands"). Standalone waits are legal as
# EventSemaphore instructions, so hoist all but the last embedded wait.
def _fix_bir(bir: dict) -> dict:
    counter = [0]
    for fn in bir.get("functions", []):
        for bb in fn.get("blocks", []):
            out = []
            for ins in bb.get("instructions", []):
                si = ins.get("sync_info") or {}
                waits = si.get("on_wait") or []
                if len(waits) > 1 and ins.get("engine") not in (None, "Unassigned"):
                    for w in waits[:-1]:
                        counter[0] += 1
                        ev = {
                            "engine": ins["engine"],
                            "ins": [],
                            "name": f"hoistwait_{counter[0]}",
                            "opcode": "EventSemaphore",
                            "outs": [],
                            "sync_info": {"on_update": [], "on_wait": [w]},
                        }
                        if "debug" in ins:
                            ev["debug"] = ins["debug"]
                        out.append(ev)
                    si["on_wait"] = [waits[-1]]
                out.append(ins)
            bb["instructions"] = out
    return bir


def _install_waitfix(nc):
    orig = nc.to_json_bytes

    def patched():
        return json.dumps(_fix_bir(json.loads(orig()))).encode()

    nc.to_json_bytes = patched


# ---------------------------------------------------------------------------
def _build():
    import concourse.bass as bass
    import concourse.tile as tile
    from concourse import mybir

    f16, f32 = mybir.dt.float16, mybir.dt.float32
    Exp = mybir.ActivationFunctionType.Exp
    Ident = mybir.ActivationFunctionType.Identity

    nc = bass.Bass()
    ev = nc.dram_tensor("ev", [128, CT, NQ], f16, kind="ExternalInput")
    img = nc.dram_tensor("img", [128, CT, NQ], f16, kind="ExternalInput")
    wq = nc.dram_tensor("wq", [128, CT, C], f16, kind="ExternalInput")
    wk = nc.dram_tensor("wk", [128, CT, C], f16, kind="ExternalInput")
    wv = nc.dram_tensor("wv", [128, CT, C], f16, kind="ExternalInput")
    bq = nc.dram_tensor("bq", [128, CT], f32, kind="ExternalInput")
    bk = nc.dram_tensor("bk", [128, CT], f32, kind="ExternalInput")
    bv = nc.dram_tensor("bv", [C], f16, kind="ExternalInput")
    out = nc.dram_tensor("out", [Q4, 128, C], f32, kind="ExternalOutput")

    PAIRS = [[0, 1], [2, 3], [4, 5], [6, 7]]

    with tile.TileContext(nc) as tc:
        with (
            tc.tile_pool(name="w", bufs=1) as wpool,
            tc.tile_pool(name="x", bufs=1) as xpool,
            tc.tile_pool(name="kv", bufs=1) as kvpool,
            tc.tile_pool(name="acc", bufs=1) as accpool,
            tc.tile_pool(name="small", bufs=1) as small,
            tc.tile_pool(name="wsum", bufs=2) as wsum,
            tc.tile_pool(name="work", bufs=2) as work,
            tc.tile_pool(name="dram", bufs=1, space="DRAM") as dpool,
            tc.tile_pool(name="psA", bufs=4, space="PSUM") as psA,
            tc.tile_pool(name="psB", bufs=3, space="PSUM") as psB,
        ):
            # ---- input DMAs (order matters: k-proj inputs first) --------
            img_sb = xpool.tile([128, CT, NQ], f16, name="img_sb", tag="x0")
            nc.sync.dma_start(out=img_sb[:], in_=img[:])
            wk_sb = wpool.tile([128, CT, C], f16, name="wk_sb", tag="w0")
            nc.sync.dma_start(out=wk_sb[:], in_=wk[:])
            bk_sb = small.tile([128, CT], f32, name="bk_sb", tag="bk")
            nc.sync.dma_start(out=bk_sb[:], in_=bk[:])
            bq_sb = small.tile([128, CT], f32, name="bq_sb", tag="bq")
            nc.sync.dma_start(out=bq_sb[:], in_=bq[:])
            bv_sb = small.tile([128, C], f16, name="bv_sb", tag="bv")
            nc.sync.dma_start(out=bv_sb[:], in_=bv[None, :].to_broadcast((128, C)))
            wv_sb = wpool.tile([128, CT, C], f16, name="wv_sb", tag="w1")
            nc.sync.dma_start(out=wv_sb[:], in_=wv[:])
            ev_sb = xpool.tile([128, CT, NQ], f16, name="ev_sb", tag="x1")
            nc.sync.dma_start(out=ev_sb[:], in_=ev[:])
            # wq reuses wk's slot -> its DMA fires once k-proj drains wk.
            # Issued after the biases so it can't head-of-line-block them.
            wq_sb = wpool.tile([128, CT, C], f16, name="wq_sb", tag="w0")
            nc.sync.dma_start(out=wq_sb[:], in_=wq[:])

            ones_sb = small.tile([128, 1], f16, name="ones_sb", tag="ones")
            nc.vector.memset(ones_sb[:], 1.0)

            pid = nc.sync.partition_id()

            kh_sb = kvpool.tile([128, CT, NQ], f16, name="kh_sb", tag="kh")
            vh_sb = kvpool.tile([128, KT2, C], f16, name="vh_sb", tag="vh")
            q_sb = kvpool.tile([128, CT, NQ], f16, name="q_sb", tag="q")
            k_rem = kvpool.tile([128, CT, NQ], f16, name="k_rem", tag="kr")
            v_rem = kvpool.tile([128, KT2, C], f16, name="v_rem", tag="vr")

            kst = dpool.tile([128, CT, NQ], f16, name="kst", tag="kst")
            vst = dpool.tile([128, KT2, C], f16, name="vst", tag="vst")
            kg = dpool.tile([2, 128, CT, NQ], f16, name="kg", tag="kg")
            vg = dpool.tile([2, 128, KT2, C], f16, name="vg", tag="vg")

            # ---- k^T half projection + k-gather -------------------------
            for co in range(CT):
                pk0 = psA.tile([128, 512], f32, name=f"pk0_{co}", tag="A")
                pk1 = psA.tile([128, 512], f32, name=f"pk1_{co}", tag="A")
                for t in range(CT):
                    st = wk_sb[:, t, co * 128:(co + 1) * 128]
                    nc.tensor.matmul(pk0, st, img_sb[:, t, 0:512],
                                     start=(t == 0), stop=(t == CT - 1))
                    nc.tensor.matmul(pk1, st, img_sb[:, t, 512:1024],
                                     start=(t == 0), stop=(t == CT - 1))
                nc.scalar.activation(kh_sb[:, co, 0:512], pk0, Ident,
                                     bias=bk_sb[:, co:co + 1])
                nc.scalar.activation(kh_sb[:, co, 512:1024], pk1, Ident,
                                     bias=bk_sb[:, co:co + 1])
            nc.sync.dma_start(out=kst[:], in_=kh_sb[:])
            nc.gpsimd.collective_compute(
                "AllGather", mybir.AluOpType.bypass, replica_groups=PAIRS,
                ins=[kst[:]], outs=[kg[:]],
            )

            # ---- v half projection + v-gather ---------------------------
            for nr in range(KT2):
                pv0 = psA.tile([128, 512], f32, name=f"pv0_{nr}", tag="A")
                pv1 = psA.tile([128, 512], f32, name=f"pv1_{nr}", tag="A")
                for t in range(CT):
                    st = img_sb[:, t, nr * 128:(nr + 1) * 128]
                    nc.tensor.matmul(pv0, st, wv_sb[:, t, 0:512],
                                     start=(t == 0), stop=(t == CT - 1))
                    nc.tensor.matmul(pv1, st, wv_sb[:, t, 512:1024],
                                     start=(t == 0), stop=(t == CT - 1))
                nc.vector.tensor_add(vh_sb[:, nr, 0:512], pv0, bv_sb[:, 0:512])
                nc.vector.tensor_add(vh_sb[:, nr, 512:1024], pv1,
                                     bv_sb[:, 512:1024])
            nc.sync.dma_start(out=vst[:], in_=vh_sb[:])
            nc.gpsimd.collective_compute(
                "AllGather", mybir.AluOpType.bypass, replica_groups=PAIRS,
                ins=[vst[:]], outs=[vg[:]],
            )

            # ---- partner-slot readback (fires when gathers land) --------
            with tc.If((pid % 2) == 0) as cif:
                nc.sync.dma_start(out=k_rem[:], in_=kg[1])
                nc.sync.dma_start(out=v_rem[:], in_=vg[1])
            with cif.Else():
                nc.sync.dma_start(out=k_rem[:], in_=kg[0])
                nc.sync.dma_start(out=v_rem[:], in_=vg[0])

            # ---- q^T projection (overlaps k-gather) ---------------------
            for co in range(CT):
                pq0 = psA.tile([128, 512], f32, name=f"pq0_{co}", tag="A")
                pq1 = psA.tile([128, 512], f32, name=f"pq1_{co}", tag="A")
                for t in range(CT):
                    st = wq_sb[:, t, co * 128:(co + 1) * 128]
                    nc.tensor.matmul(pq0, st, ev_sb[:, t, 0:512],
                                     start=(t == 0), stop=(t == CT - 1))
                    nc.tensor.matmul(pq1, st, ev_sb[:, t, 512:1024],
                                     start=(t == 0), stop=(t == CT - 1))
                nc.scalar.activation(q_sb[:, co, 0:512], pq0, Ident,
                                     bias=bq_sb[:, co:co + 1])
                nc.scalar.activation(q_sb[:, co, 512:1024], pq1, Ident,
                                     bias=bq_sb[:, co:co + 1])

            # ---- attention helpers --------------------------------------
            def scores_pass(k_src, exp_dst, lbl):
                for kt in range(KT2):
                    ps0 = psB.tile([128, 512], f32, name=f"s0{lbl}{kt}", tag="B")
                    ps1 = psB.tile([128, 512], f32, name=f"s1{lbl}{kt}", tag="B")
                    for t in range(CT):
                        st = k_src[:, t, kt * 128:(kt + 1) * 128]
                        nc.tensor.matmul(ps0, st, q_sb[:, t, 0:512],
                                         start=(t == 0), stop=(t == CT - 1))
                        nc.tensor.matmul(ps1, st, q_sb[:, t, 512:1024],
                                         start=(t == 0), stop=(t == CT - 1))
                    nc.scalar.activation(exp_dst[:, kt, 0:512], ps0, Exp,
                                         scale=float(SCALE))
                    nc.scalar.activation(exp_dst[:, kt, 512:1024], ps1, Exp,
                                         scale=float(SCALE))

            def kt_reduce(exp_src, red_dst):
                nc.vector.tensor_add(red_dst[:], exp_src[:, 0, :],
                                     exp_src[:, 1, :])
                for kt in range(2, KT2):
                    nc.vector.tensor_add(red_dst[:], red_dst[:],
                                         exp_src[:, kt, :])

            # ---- pass A: attention vs LOCAL keys (overlaps gathers) -----
            exp_loc = xpool.tile([128, KT2, NQ], f16, name="exp_loc", tag="x0")
            scores_pass(kh_sb, exp_loc, "l")
            red_loc = wsum.tile([128, NQ], f16, name="red_loc", tag="red")
            kt_reduce(exp_loc, red_loc)

            out_acc = accpool.tile([128, Q4, C], f16, name="out_acc", tag="oacc")
            sums_loc = small.tile([128, Q4], f32, name="sums_loc", tag="sloc")
            for q4 in range(Q4):
                po0 = psA.tile([128, 512], f32, name=f"pl0_{q4}", tag="A")
                po1 = psA.tile([128, 512], f32, name=f"pl1_{q4}", tag="A")
                for kt in range(KT2):
                    st = exp_loc[:, kt, q4 * 128:(q4 + 1) * 128]
                    nc.tensor.matmul(po0, st, vh_sb[:, kt, 0:512],
                                     start=(kt == 0), stop=(kt == KT2 - 1))
                    nc.tensor.matmul(po1, st, vh_sb[:, kt, 512:1024],
                                     start=(kt == 0), stop=(kt == KT2 - 1))
                nc.scalar.copy(out_acc[:, q4, 0:512], po0)
                nc.scalar.copy(out_acc[:, q4, 512:1024], po1)
                pss = psA.tile([128, 1], f32, name=f"psl_{q4}", tag="S", bufs=1)
                nc.tensor.matmul(pss, red_loc[:, q4 * 128:(q4 + 1) * 128],
                                 ones_sb[:], start=True, stop=True)
                nc.scalar.copy(sums_loc[:, q4:q4 + 1], pss)

            # ---- pass B: attention vs REMOTE keys -----------------------
            exp_rem = xpool.tile([128, KT2, NQ], f16, name="exp_rem", tag="x1")
            scores_pass(k_rem, exp_rem, "r")
            red_rem = wsum.tile([128, NQ], f16, name="red_rem", tag="red")
            kt_reduce(exp_rem, red_rem)

            for q4 in range(Q4):
                po0 = psA.tile([128, 512], f32, name=f"pr0_{q4}", tag="A")
                po1 = psA.tile([128, 512], f32, name=f"pr1_{q4}", tag="A")
                for kt in range(KT2):
                    st = exp_rem[:, kt, q4 * 128:(q4 + 1) * 128]
                    nc.tensor.matmul(po0, st, v_rem[:, kt, 0:512],
                                     start=(kt == 0), stop=(kt == KT2 - 1))
                    nc.tensor.matmul(po1, st, v_rem[:, kt, 512:1024],
                                     start=(kt == 0), stop=(kt == KT2 - 1))
                pss = psA.tile([128, 1], f32, name=f"psr_{q4}", tag="S", bufs=1)
                nc.tensor.matmul(pss, red_rem[:, q4 * 128:(q4 + 1) * 128],
                                 ones_sb[:], start=True, stop=True)
                stot = work.tile([128, 1], f32, name=f"stot_{q4}", tag="stot")
                nc.vector.tensor_add(stot[:], pss, sums_loc[:, q4:q4 + 1])
                recip = work.tile([128, 1], f32, name=f"recip_{q4}", tag="recip")
                nc.vector.reciprocal(recip[:], stot[:])
                o_sb = work.tile([128, C], f32, name=f"o_{q4}", tag="o")
                o2_sb = work.tile([128, C], f32, name=f"o2_{q4}", tag="o2")
                nc.vector.tensor_add(o_sb[:, 0:512], po0, out_acc[:, q4, 0:512])
                nc.vector.tensor_add(o_sb[:, 512:1024], po1,
                                     out_acc[:, q4, 512:1024])
                nc.scalar.mul(o2_sb[:, 0:512], o_sb[:, 0:512], recip[:])
                nc.scalar.mul(o2_sb[:, 512:1024], o_sb[:, 512:1024], recip[:])
                nc.sync.dma_start(out=out[q4], in_=o2_sb[:])

    _install_waitfix(nc)
    return nc


def _get_nc():
    if "nc" not in _CACHE:
        _CACHE["nc"] = _build()
    return _CACHE["nc"]


def _tile_cp(x16):
    """[C, n] fp16 -> [128, CT, n] with c = t*128 + p."""
    n = x16.shape[1]
    return np.ascontiguousarray(
        x16.reshape(CT, 128, n).transpose(1, 0, 2))


def run(inputs, trace=False, trace_cores=None):
    from concourse.bass_utils import run_bass_kernel_spmd

    event_f = np.asarray(inputs["event_f"], dtype=np.float32)
    img_f = np.asarray(inputs["img_f"], dtype=np.float32)
    Wq = _tile_cp(np.asarray(inputs["Wq"], dtype=np.float32).astype(np.float16))
    Wk = _tile_cp(np.asarray(inputs["Wk"], dtype=np.float32).astype(np.float16))
    Wv = _tile_cp(np.asarray(inputs["Wv"], dtype=np.float32).astype(np.float16))
    bq = np.ascontiguousarray(
        np.asarray(inputs["bq"], dtype=np.float32).reshape(CT, 128).T)
    bk = np.ascontiguousarray(
        np.asarray(inputs["bk"], dtype=np.float32).reshape(CT, 128).T)
    bv = np.asarray(inputs["bv"], dtype=np.float32).astype(np.float16)

    in_maps = []
    for core in range(8):
        b, h = core // 2, core % 2
        ev_t = _tile_cp(np.ascontiguousarray(
            event_f[b, h * NQ:(h + 1) * NQ, :].T).astype(np.float16))
        img_t = _tile_cp(np.ascontiguousarray(
            img_f[b].T[:, h * NQ:(h + 1) * NQ]).astype(np.float16))
        in_maps.append({
            "ev": ev_t, "img": img_t,
            "wq": Wq, "wk": Wk, "wv": Wv,
            "bq": bq, "bk": bk, "bv": bv,
        })

    nc = _get_nc()
    res = run_bass_kernel_spmd(
        nc, in_maps, list(range(8)), trace=trace,
        **({"trace_cores": trace_cores} if trace_cores else {}),
    )
    full = np.empty((B, N, C), dtype=np.float32)
    for core in range(8):
        b, h = core // 2, core % 2
        full[b, h * NQ:(h + 1) * NQ, :] = res.results[core]["out"].reshape(NQ, C)
    return full, res


def kernel(**inputs) -> np.ndarray:
    full, _ = run(inputs, trace=False)
    return full


# revision 15
# speedup vs baseline: 1.0166x; 1.0166x over previous
"""Cross-attention kernel for Trainium2, SPMD over 8 NeuronCores.

Problem: B=4, N=2048, C=1024 fp32.
  q = event_f @ Wq + bq ; k = img_f @ Wk + bk ; v = img_f @ Wv + bv
  out = softmax(q k^T / sqrt(C)) v

Sharding: core i = (batch b = i//2, half h = i%2). Each core projects q^T for
its 1024 query rows and k^T/v for its 1024 KEY rows, exchanges the k/v halves
with its pair partner (two early pairwise AllGathers, k first), and overlaps
the exchange with attention against its LOCAL keys.  The partner half is then
read back (tc.If on partition_id&1 picks the partner slot) and attention over
the REMOTE keys finishes the job:

  out[q,:] = (sum_loc p_l v_l + sum_rem p_r v_r) / (S_l + S_r)

Softmax is key-order invariant, so local/remote key splitting is safe.  No
max-subtraction (logits are N(0,1)); exp in fp16.

Layouts (all fat-row, 16KB contiguous per partition, for large DMA packets):
  host ships x^T / W tiled as [128p, 8t, 1024] fp16 with c_in = t*128+p.
  - q^T/k^T[c,n]: stationary = W[:, t, co*128:+128], moving = x^T tiles
  - v[n,c]:       stationary = img^T[:, t, nr*128:+128], moving = Wv
  - s^T[k,q]:     stationary = k^T[:, t, kt*128:+128], moving = q^T
  - out[q,c]:     stationary = exp[:, kt, q4*128:+128], moving = v
  - sums: DVE reduces exp over kt -> red[128,1024]; one matmul per q4 block
    (red_blk^T @ ones) gives per-query partial sums.
All matmul operands fp16, PSUM fp32, output fp32.
"""

import json

import numpy as np

B, N, C = 4, 2048, 1024
NQ = N // 2          # query rows per core
CT = C // 128        # contraction tiles
KT2 = NQ // 128      # key tiles per half
Q4 = NQ // 128       # query tiles
SCALE = 1.0 / np.sqrt(C)

_CACHE = {}


# ---------------------------------------------------------------------------
# Walrus in this container rejects >1 embedded sem-wait per instruction
# ("Too many sync wait commands"). Standalone waits are legal as
# EventSemaphore instructions, so hoist all but the last embedded wait.
def _fix_bir(bir: dict) -> dict:
    counter = [0]
    for fn in bir.get("functions", []):
        for bb in fn.get("blocks", []):
            out = []
            for ins in bb.get("instructions", []):
                si = ins.get("sync_info") or {}
                waits = si.get("on_wait") or []
                if len(waits) > 1 and ins.get("engine") not in (None, "Unassigned"):
                    for w in waits[:-1]:
                        counter[0] += 1
                        ev = {
                            "engine": ins["engine"],
                            "ins": [],
                            "name": f"hoistwait_{counter[0]}",
                            "opcode": "EventSemaphore",
                            "outs": [],
                            "sync_info": {"on_update": [], "on_wait": [w]},
                        }
                        if "debug" in ins:
                            ev["debug"] = ins["debug"]
                        out.append(ev)
                    si["on_wait"] = [waits[-1]]
                out.append(ins)
            bb["instructions"] = out
    return bir


def _install_waitfix(nc):
    orig = nc.to_json_bytes

    def patched():
        return json.dumps(_fix_bir(json.loads(orig()))).encode()

    nc.to_json_bytes = patched


# ---------------------------------------------------------------------------
def _build():
    import concourse.bass as bass
    import concourse.tile as tile
    from concourse import mybir

    f16, f32 = mybir.dt.float16, mybir.dt.float32
    Exp = mybir.ActivationFunctionType.Exp
    Ident = mybir.ActivationFunctionType.Identity

    nc = bass.Bass()
    ev = nc.dram_tensor("ev", [128, CT, NQ], f16, kind="ExternalInput")
    img = nc.dram_tensor("img", [128, CT, NQ], f16, kind="ExternalInput")
    wq = nc.dram_tensor("wq", [128, CT, C], f16, kind="ExternalInput")
    wk = nc.dram_tensor("wk", [128, CT, C], f16, kind="ExternalInput")
    wv = nc.dram_tensor("wv", [128, CT, C], f16, kind="ExternalInput")
    bq = nc.dram_tensor("bq", [128, CT], f32, kind="ExternalInput")
    bk = nc.dram_tensor("bk", [128, CT], f32, kind="ExternalInput")
    bv = nc.dram_tensor("bv", [C], f16, kind="ExternalInput")
    # fp16 output (host upcasts): halves the final DMA on the critical tail
    out = nc.dram_tensor("out", [Q4, 128, C], f16, kind="ExternalOutput")

    PAIRS = [[0, 1], [2, 3], [4, 5], [6, 7]]

    with tile.TileContext(nc) as tc:
        with (
            tc.tile_pool(name="w", bufs=1) as wpool,
            tc.tile_pool(name="x", bufs=1) as xpool,
            tc.tile_pool(name="kv", bufs=1) as kvpool,
            tc.tile_pool(name="acc", bufs=1) as accpool,
            tc.tile_pool(name="small", bufs=1) as small,
            tc.tile_pool(name="wsum", bufs=2) as wsum,
            tc.tile_pool(name="work", bufs=2) as work,
            tc.tile_pool(name="dram", bufs=1, space="DRAM") as dpool,
            tc.tile_pool(name="psA", bufs=4, space="PSUM") as psA,
            tc.tile_pool(name="psB", bufs=3, space="PSUM") as psB,
        ):
            # ---- input DMAs (order matters: k-proj inputs first) --------
            img_sb = xpool.tile([128, CT, NQ], f16, name="img_sb", tag="x0")
            nc.sync.dma_start(out=img_sb[:], in_=img[:])
            wk_sb = wpool.tile([128, CT, C], f16, name="wk_sb", tag="w0")
            nc.sync.dma_start(out=wk_sb[:], in_=wk[:])
            bk_sb = small.tile([128, CT], f32, name="bk_sb", tag="bk")
            nc.sync.dma_start(out=bk_sb[:], in_=bk[:])
            bq_sb = small.tile([128, CT], f32, name="bq_sb", tag="bq")
            nc.sync.dma_start(out=bq_sb[:], in_=bq[:])
            bv_sb = small.tile([128, C], f16, name="bv_sb", tag="bv")
            nc.sync.dma_start(out=bv_sb[:], in_=bv[None, :].to_broadcast((128, C)))
            wv_sb = wpool.tile([128, CT, C], f16, name="wv_sb", tag="w1")
            nc.sync.dma_start(out=wv_sb[:], in_=wv[:])
            ev_sb = xpool.tile([128, CT, NQ], f16, name="ev_sb", tag="x1")
            nc.sync.dma_start(out=ev_sb[:], in_=ev[:])
            # wq reuses wk's slot -> its DMA fires once k-proj drains wk.
            # Issued after the biases so it can't head-of-line-block them.
            wq_sb = wpool.tile([128, CT, C], f16, name="wq_sb", tag="w0")
            nc.sync.dma_start(out=wq_sb[:], in_=wq[:])

            ones_sb = small.tile([128, 1], f16, name="ones_sb", tag="ones")
            nc.vector.memset(ones_sb[:], 1.0)

            pid = nc.sync.partition_id()

            kh_sb = kvpool.tile([128, CT, NQ], f16, name="kh_sb", tag="kh")
            vh_sb = kvpool.tile([128, KT2, C], f16, name="vh_sb", tag="vh")
            q_sb = kvpool.tile([128, CT, NQ], f16, name="q_sb", tag="q")
            k_rem = kvpool.tile([128, CT, NQ], f16, name="k_rem", tag="kr")
            v_rem = kvpool.tile([128, KT2, C], f16, name="v_rem", tag="vr")

            kst = dpool.tile([128, CT, NQ], f16, name="kst", tag="kst")
            vst = dpool.tile([128, KT2, C], f16, name="vst", tag="vst")
            kg = dpool.tile([2, 128, CT, NQ], f16, name="kg", tag="kg")
            vg = dpool.tile([2, 128, KT2, C], f16, name="vg", tag="vg")

            # ---- k^T half projection + k-gather -------------------------
            for co in range(CT):
                pk0 = psA.tile([128, 512], f32, name=f"pk0_{co}", tag="A")
                pk1 = psA.tile([128, 512], f32, name=f"pk1_{co}", tag="A")
                for t in range(CT):
                    st = wk_sb[:, t, co * 128:(co + 1) * 128]
                    nc.tensor.matmul(pk0, st, img_sb[:, t, 0:512],
                                     start=(t == 0), stop=(t == CT - 1))
                    nc.tensor.matmul(pk1, st, img_sb[:, t, 512:1024],
                                     start=(t == 0), stop=(t == CT - 1))
                nc.scalar.activation(kh_sb[:, co, 0:512], pk0, Ident,
                                     bias=bk_sb[:, co:co + 1])
                nc.scalar.activation(kh_sb[:, co, 512:1024], pk1, Ident,
                                     bias=bk_sb[:, co:co + 1])
            nc.sync.dma_start(out=kst[:], in_=kh_sb[:])
            nc.gpsimd.collective_compute(
                "AllGather", mybir.AluOpType.bypass, replica_groups=PAIRS,
                ins=[kst[:]], outs=[kg[:]],
            )

            # ---- v half projection + v-gather ---------------------------
            for nr in range(KT2):
                pv0 = psA.tile([128, 512], f32, name=f"pv0_{nr}", tag="A")
                pv1 = psA.tile([128, 512], f32, name=f"pv1_{nr}", tag="A")
                for t in range(CT):
                    st = img_sb[:, t, nr * 128:(nr + 1) * 128]
                    nc.tensor.matmul(pv0, st, wv_sb[:, t, 0:512],
                                     start=(t == 0), stop=(t == CT - 1))
                    nc.tensor.matmul(pv1, st, wv_sb[:, t, 512:1024],
                                     start=(t == 0), stop=(t == CT - 1))
                nc.vector.tensor_add(vh_sb[:, nr, 0:512], pv0, bv_sb[:, 0:512])
                nc.vector.tensor_add(vh_sb[:, nr, 512:1024], pv1,
                                     bv_sb[:, 512:1024])
            nc.sync.dma_start(out=vst[:], in_=vh_sb[:])
            nc.gpsimd.collective_compute(
                "AllGather", mybir.AluOpType.bypass, replica_groups=PAIRS,
                ins=[vst[:]], outs=[vg[:]],
            )

            # ---- partner-slot readback (fires when gathers land) --------
            with tc.If((pid % 2) == 0) as cif:
                nc.sync.dma_start(out=k_rem[:], in_=kg[1])
                nc.sync.dma_start(out=v_rem[:], in_=vg[1])
            with cif.Else():
                nc.sync.dma_start(out=k_rem[:], in_=kg[0])
                nc.sync.dma_start(out=v_rem[:], in_=vg[0])

            # ---- q^T projection (overlaps k-gather) ---------------------
            for co in range(CT):
                pq0 = psA.tile([128, 512], f32, name=f"pq0_{co}", tag="A")
                pq1 = psA.tile([128, 512], f32, name=f"pq1_{co}", tag="A")
                for t in range(CT):
                    st = wq_sb[:, t, co * 128:(co + 1) * 128]
                    nc.tensor.matmul(pq0, st, ev_sb[:, t, 0:512],
                                     start=(t == 0), stop=(t == CT - 1))
                    nc.tensor.matmul(pq1, st, ev_sb[:, t, 512:1024],
                                     start=(t == 0), stop=(t == CT - 1))
                nc.scalar.activation(q_sb[:, co, 0:512], pq0, Ident,
                                     bias=bq_sb[:, co:co + 1])
                nc.scalar.activation(q_sb[:, co, 512:1024], pq1, Ident,
                                     bias=bq_sb[:, co:co + 1])

            # ---- attention helpers --------------------------------------
            def scores_pass(k_src, exp_dst, lbl):
                for kt in range(KT2):
                    ps0 = psB.tile([128, 512], f32, name=f"s0{lbl}{kt}", tag="B")
                    ps1 = psB.tile([128, 512], f32, name=f"s1{lbl}{kt}", tag="B")
                    for t in range(CT):
                        st = k_src[:, t, kt * 128:(kt + 1) * 128]
                        nc.tensor.matmul(ps0, st, q_sb[:, t, 0:512],
                                         start=(t == 0), stop=(t == CT - 1))
                        nc.tensor.matmul(ps1, st, q_sb[:, t, 512:1024],
                                         start=(t == 0), stop=(t == CT - 1))
                    nc.scalar.activation(exp_dst[:, kt, 0:512], ps0, Exp,
                                         scale=float(SCALE))
                    nc.scalar.activation(exp_dst[:, kt, 512:1024], ps1, Exp,
                                         scale=float(SCALE))

            def kt_reduce(exp_src, red_dst):
                nc.vector.tensor_add(red_dst[:], exp_src[:, 0, :],
                                     exp_src[:, 1, :])
                for kt in range(2, KT2):
                    nc.vector.tensor_add(red_dst[:], red_dst[:],
                                         exp_src[:, kt, :])

            # ---- pass A: attention vs LOCAL keys (overlaps gathers) -----
            exp_loc = xpool.tile([128, KT2, NQ], f16, name="exp_loc", tag="x0")
            scores_pass(kh_sb, exp_loc, "l")
            red_loc = wsum.tile([128, NQ], f16, name="red_loc", tag="red")
            kt_reduce(exp_loc, red_loc)

            out_acc = accpool.tile([128, Q4, C], f16, name="out_acc", tag="oacc")
            sums_loc = small.tile([128, Q4], f32, name="sums_loc", tag="sloc")
            for q4 in range(Q4):
                po0 = psA.tile([128, 512], f32, name=f"pl0_{q4}", tag="A")
                po1 = psA.tile([128, 512], f32, name=f"pl1_{q4}", tag="A")
                for kt in range(KT2):
                    st = exp_loc[:, kt, q4 * 128:(q4 + 1) * 128]
                    nc.tensor.matmul(po0, st, vh_sb[:, kt, 0:512],
                                     start=(kt == 0), stop=(kt == KT2 - 1))
                    nc.tensor.matmul(po1, st, vh_sb[:, kt, 512:1024],
                                     start=(kt == 0), stop=(kt == KT2 - 1))
                nc.scalar.copy(out_acc[:, q4, 0:512], po0)
                nc.scalar.copy(out_acc[:, q4, 512:1024], po1)
                pss = psA.tile([128, 1], f32, name=f"psl_{q4}", tag="S", bufs=1)
                nc.tensor.matmul(pss, red_loc[:, q4 * 128:(q4 + 1) * 128],
                                 ones_sb[:], start=True, stop=True)
                nc.scalar.copy(sums_loc[:, q4:q4 + 1], pss)

            # ---- pass B: attention vs REMOTE keys -----------------------
            exp_rem = xpool.tile([128, KT2, NQ], f16, name="exp_rem", tag="x1")
            scores_pass(k_rem, exp_rem, "r")
            red_rem = wsum.tile([128, NQ], f16, name="red_rem", tag="red")
            kt_reduce(exp_rem, red_rem)

            for q4 in range(Q4):
                po0 = psA.tile([128, 512], f32, name=f"pr0_{q4}", tag="A")
                po1 = psA.tile([128, 512], f32, name=f"pr1_{q4}", tag="A")
                for kt in range(KT2):
                    st = exp_rem[:, kt, q4 * 128:(q4 + 1) * 128]
                    nc.tensor.matmul(po0, st, v_rem[:, kt, 0:512],
                                     start=(kt == 0), stop=(kt == KT2 - 1))
                    nc.tensor.matmul(po1, st, v_rem[:, kt, 512:1024],
                                     start=(kt == 0), stop=(kt == KT2 - 1))
                pss = psA.tile([128, 1], f32, name=f"psr_{q4}", tag="S", bufs=1)
                nc.tensor.matmul(pss, red_rem[:, q4 * 128:(q4 + 1) * 128],
                                 ones_sb[:], start=True, stop=True)
                stot = work.tile([128, 1], f32, name=f"stot_{q4}", tag="stot")
                nc.vector.tensor_add(stot[:], pss, sums_loc[:, q4:q4 + 1])
                recip = work.tile([128, 1], f32, name=f"recip_{q4}", tag="recip")
                nc.vector.reciprocal(recip[:], stot[:])
                o_sb = work.tile([128, C], f32, name=f"o_{q4}", tag="o")
                o2_sb = work.tile([128, C], f16, name=f"o2_{q4}", tag="o2")
                nc.vector.tensor_add(o_sb[:, 0:512], po0, out_acc[:, q4, 0:512])
                nc.vector.tensor_add(o_sb[:, 512:1024], po1,
                                     out_acc[:, q4, 512:1024])
                nc.scalar.mul(o2_sb[:, 0:512], o_sb[:, 0:512], recip[:])
                nc.scalar.mul(o2_sb[:, 512:1024], o_sb[:, 512:1024], recip[:])
                nc.sync.dma_start(out=out[q4], in_=o2_sb[:])

    _install_waitfix(nc)
    return nc


def _get_nc():
    if "nc" not in _CACHE:
        _CACHE["nc"] = _build()
    return _CACHE["nc"]


def _tile_cp(x16):
    """[C, n] fp16 -> [128, CT, n] with c = t*128 + p."""
    n = x16.shape[1]
    return np.ascontiguousarray(
        x16.reshape(CT, 128, n).transpose(1, 0, 2))


def run(inputs, trace=False, trace_cores=None):
    from concourse.bass_utils import run_bass_kernel_spmd

    event_f = np.asarray(inputs["event_f"], dtype=np.float32)
    img_f = np.asarray(inputs["img_f"], dtype=np.float32)
    Wq = _tile_cp(np.asarray(inputs["Wq"], dtype=np.float32).astype(np.float16))
    Wk = _tile_cp(np.asarray(inputs["Wk"], dtype=np.float32).astype(np.float16))
    Wv = _tile_cp(np.asarray(inputs["Wv"], dtype=np.float32).astype(np.float16))
    bq = np.ascontiguousarray(
        np.asarray(inputs["bq"], dtype=np.float32).reshape(CT, 128).T)
    bk = np.ascontiguousarray(
        np.asarray(inputs["bk"], dtype=np.float32).reshape(CT, 128).T)
    bv = np.asarray(inputs["bv"], dtype=np.float32).astype(np.float16)

    in_maps = []
    for core in range(8):
        b, h = core // 2, core % 2
        ev_t = _tile_cp(np.ascontiguousarray(
            event_f[b, h * NQ:(h + 1) * NQ, :].T).astype(np.float16))
        img_t = _tile_cp(np.ascontiguousarray(
            img_f[b].T[:, h * NQ:(h + 1) * NQ]).astype(np.float16))
        in_maps.append({
            "ev": ev_t, "img": img_t,
            "wq": Wq, "wk": Wk, "wv": Wv,
            "bq": bq, "bk": bk, "bv": bv,
        })

    nc = _get_nc()
    res = run_bass_kernel_spmd(
        nc, in_maps, list(range(8)), trace=trace,
        **({"trace_cores": trace_cores} if trace_cores else {}),
    )
    full = np.empty((B, N, C), dtype=np.float32)
    for core in range(8):
        b, h = core // 2, core % 2
        full[b, h * NQ:(h + 1) * NQ, :] = res.results[core]["out"].reshape(NQ, C)
    return full, res


def kernel(**inputs) -> np.ndarray:
    full, _ = run(inputs, trace=False)
    return full
